# revision 26
# baseline (speedup 1.0000x reference)
"""Additive (Bahdanau) attention kernel for Trainium2, 8 NeuronCores.

Problem: B=4, H=16, L=8192, D=64 (fp32)
    e1 = q @ Wa_w.T + Wa_b ; e2 = k @ Ua_w.T + Ua_b
    t  = tanh(e1 + e2)
    e  = t @ va_w[0] + va_b          (va_b dropped: softmax shift-invariant)
    e  = where(mask == 0, -1e4, e)   (additive -1e4 bias before exp)
    alpha = softmax(e, axis=-1)      (over L)
    out = alpha[..., None] * v

Sharding: 64 independent (b, h) slices -> 8 per core, no collectives.

Per-slice layout: l = p * (L/128) + j with p = SBUF partition, j = tile
column; q/k/v live as [128, J, 64] (contiguous per partition), q/k/v are
cast to bf16 by the SWDGE DMA.  j-tiles are processed in PAIRS (transpose
q[:, j0:j0+2, :] [128,128] on TensorE -> bf16 psum rows (jj,d)), pairs
are processed in GROUPS of 4 to batch the weight matmuls (N=512) and
tanh.  Block-diagonal [WaT|WaT], [UaT|UaT] bf16 weights give e1+e2 for
both tiles of a pair in one K=128 contraction; tanh+bias on ScalarE ->
bf16; per-pair score matmul with tT stationary and a 2-column block-va
moving operand yields both score columns in natural [128, 1] layout.

Softmax: additive -1e4 mask bias (DVE), exp with fused per-row
accumulate (ScalarE accum_out), cross-partition sum via a ones-matmul on
TensorE (keeps GpSimd free for SWDGE descriptor generation), reciprocal
straight from PSUM, then a free-dim broadcast multiply with v on DVE.
The softmax/v-scale/store of each slice is software-pipelined into the
next slice's group loop (halves at g4/g6) so no engine blocks in-line.

Steady state is HBM-roofline bound (~7.1 MB HBM per slice).  All engine
queues are kept clear of long waits: consts are pre-cast to bf16 on the
host and loaded via HWDGE, the mask is one batched HWDGE load, v has 3
buffers so the SWDGE stream never stalls on a late v_sb recycle, and
PSUM->SBUF transpose copies alternate between DVE and ScalarE.
"""

import numpy as np
import ml_dtypes
from contextlib import ExitStack

import concourse.bass as bass
import concourse.tile as tile
from concourse import bacc, mybir
from concourse.bass_utils import run_bass_kernel_spmd

B, H, L, D = 4, 16, 8192, 64
N_CORES = 8
SLICES_PER_CORE = (B * H) // N_CORES
P = 128

F32 = mybir.dt.float32
BF16 = mybir.dt.bfloat16
I32 = mybir.dt.int32


def build_bass(n_slices=SLICES_PER_CORE, seq=L, pipeline=True):
    JT = seq // P            # j-columns per slice
    NPAIR = JT // 2          # tile pairs per slice
    NGRP = NPAIR // 4        # groups of 4 pairs
    assert NPAIR % 4 == 0

    nc = bacc.Bacc(target_bir_lowering=False)
    q_ext = nc.declare_dram_parameter("q", [n_slices, seq, D], F32, isOutput=False)
    k_ext = nc.declare_dram_parameter("k", [n_slices, seq, D], F32, isOutput=False)
    v_ext = nc.declare_dram_parameter("v", [n_slices, seq, D], F32, isOutput=False)
    # additive mask bias, precomputed on host in on-chip layout [p, s, j]
    # (a device-side gather of mask[s, (p j)] would be a 256B-descriptor
    # strided DMA that takes ~23us and poisons the DVE queue)
    mb_ext = nc.declare_dram_parameter("maskb", [P, n_slices * (seq // P)], F32,
                                       isOutput=False)
    wb_ext = nc.declare_dram_parameter("wblk", [P, P], BF16, isOutput=False)
    ub_ext = nc.declare_dram_parameter("ublk", [P, P], BF16, isOutput=False)
    b2_ext = nc.declare_dram_parameter("bias2", [P, 1], F32, isOutput=False)
    va_ext = nc.declare_dram_parameter("vablk", [P, 2], BF16, isOutput=False)
    id_ext = nc.declare_dram_parameter("ident", [P, P], BF16, isOutput=False)
    on_ext = nc.declare_dram_parameter("ones", [P, P], BF16, isOutput=False)
    out_ext = nc.declare_dram_parameter("out", [n_slices, seq, D], BF16,
                                        isOutput=True)

    with tile.TileContext(nc) as tc, ExitStack() as ctx:
        consts = ctx.enter_context(tc.tile_pool(name="consts", bufs=1))
        bigq = ctx.enter_context(tc.tile_pool(name="bigq", bufs=2))
        bigk = ctx.enter_context(tc.tile_pool(name="bigk", bufs=2))
        bigv = ctx.enter_context(tc.tile_pool(name="bigv", bufs=3))
        bigo = ctx.enter_context(tc.tile_pool(name="bigo", bufs=2))
        chunks = ctx.enter_context(tc.tile_pool(name="chunks", bufs=3))
        smalls = ctx.enter_context(tc.tile_pool(name="smalls", bufs=2))
        ps_t = ctx.enter_context(tc.tile_pool(name="ps_t", bufs=3, space="PSUM"))
        ps_e = ctx.enter_context(tc.tile_pool(name="ps_e", bufs=3, space="PSUM"))
        ps_sc = ctx.enter_context(tc.tile_pool(name="ps_sc", bufs=2, space="PSUM"))

        # chunk splits per slice: slice 0 starts fine-grained so the first
        # transposes can begin ~4us after the first descriptors, later
        # slices use big 2MB-HBM chunks for best SDMA efficiency
        jq8 = max(JT // 8, 1)
        first_splits = [(0, jq8), (jq8, 2 * jq8), (2 * jq8, JT // 2),
                        (JT // 2, JT)]
        rest_splits = [(0, JT // 2), (JT // 2, JT)]
        # the last slice tapers down so TensorE tracks chunk arrivals and
        # the final exposed compute is only ~one group deep
        last_splits = [(0, JT // 2), (JT // 2, 6 * jq8), (6 * jq8, 7 * jq8),
                       (7 * jq8, JT)]
        jc = first_splits[0][1]  # first-chunk width for the pre-issued load

        # first q/k chunk of slice 0 goes out before anything else so the
        # critical data feed starts immediately (consts ride HWDGE)
        q0_sb = bigq.tile([P, JT, D], BF16, name="q0_sb", tag="q_sb")
        k0_sb = bigk.tile([P, JT, D], BF16, name="k0_sb", tag="k_sb")
        nc.gpsimd.dma_start(q0_sb[:, 0:jc, :],
                            q_ext[0].rearrange("(p j) d -> p j d", p=P)[:, 0:jc, :])
        nc.gpsimd.dma_start(k0_sb[:, 0:jc, :],
                            k_ext[0].rearrange("(p j) d -> p j d", p=P)[:, 0:jc, :])

        # constants (pre-cast to bf16 on the host, loaded once via HWDGE)
        wblk = consts.tile([P, P], BF16)
        nc.sync.dma_start(wblk[:], wb_ext[:, :])
        ublk = consts.tile([P, P], BF16)
        nc.sync.dma_start(ublk[:], ub_ext[:, :])
        vablk = consts.tile([P, 2], BF16)
        nc.sync.dma_start(vablk[:], va_ext[:, :])
        bias2 = consts.tile([P, 1], F32)
        nc.sync.dma_start(bias2[:], b2_ext[:, :])
        ident = consts.tile([P, P], BF16)
        nc.sync.dma_start(ident[:], id_ext[:, :])
        ones = consts.tile([P, P], BF16)
        nc.sync.dma_start(ones[:], on_ext[:, :])
        # the whole additive mask bias in one contiguous load
        maskb_all = consts.tile([P, n_slices, JT], F32)
        nc.sync.dma_start(maskb_all[:],
                          mb_ext.rearrange("p (s j) -> p s j", s=n_slices))

        pending = None
        for s in range(n_slices):
            if s == 0:
                q_sb, k_sb = q0_sb, k0_sb
            else:
                q_sb = bigq.tile([P, JT, D], BF16, name="q_sb", tag="q_sb")
                k_sb = bigk.tile([P, JT, D], BF16, name="k_sb", tag="k_sb")
            if s == 0:
                splits = first_splits
            elif s == n_slices - 1:
                splits = last_splits
            else:
                splits = rest_splits
            for ci, (j0, j1) in enumerate(splits):
                if s == 0 and ci == 0:
                    continue
                js = slice(j0, j1)
                nc.gpsimd.dma_start(
                    q_sb[:, js, :],
                    q_ext[s].rearrange("(p j) d -> p j d", p=P)[:, js, :])
                nc.gpsimd.dma_start(
                    k_sb[:, js, :],
                    k_ext[s].rearrange("(p j) d -> p j d", p=P)[:, js, :])
            v_sb = bigv.tile([P, JT, D], BF16)
            nc.gpsimd.dma_start(v_sb[:], v_ext[s].rearrange("(p j) d -> p j d", p=P))

            scores_ps = ps_sc.tile([P, JT], F32)

            def make_state(last=False):
                st = {
                    "s": s, "scores_ps": scores_ps, "v_sb": v_sb,
                    "sm": smalls.tile([P, JT], F32, tag="sm", name="sm"),
                    "p_sb": smalls.tile([P, JT], F32, tag="p_sb", name="p_sb"),
                    "rowsum": smalls.tile([P, 1], F32, tag="rowsum",
                                          name="rowsum"),
                    "rowsum_bf": smalls.tile([P, 1], BF16, tag="rowsum_bf",
                                             name="rowsum_bf"),
                    "invs": smalls.tile([P, 1], F32, tag="invs", name="invs"),
                    "alpha": smalls.tile([P, JT], BF16, tag="alpha",
                                         name="alpha"),
                    "o_sb": bigo.tile([P, JT, D], BF16, tag="o_sb",
                                      name="o_sb"),
                }
                if last:
                    st["rowsum_a"] = smalls.tile([P, 1], F32, tag="rowsum_a",
                                                 name="rowsum_a")
                    st["rowsum_b"] = smalls.tile([P, 1], F32, tag="rowsum_b",
                                                 name="rowsum_b")
                return st

            # only the final slice allocates its softmax state up front (it
            # emits a half-row exp inside its own group loop); other slices
            # keep the plain emission order
            cur = (make_state(last=True)
                   if pipeline and s == n_slices - 1 and NGRP > 1 else None)

            def emit_softmax_stage(st, stage):
                """Deferred softmax/v-scale/store for a previous slice."""
                if stage == 0:
                    nc.vector.tensor_add(st["sm"][:], st["scores_ps"][:],
                                         maskb_all[:, st["s"], :])
                    nc.scalar.activation(st["p_sb"][:], st["sm"][:],
                                         mybir.ActivationFunctionType.Exp,
                                         accum_out=st["rowsum"][:])
                elif stage == 1:
                    # cross-partition sum of rowsum via ones-matmul: every
                    # partition of scores_ps[:, 0:1] gets the full-row total
                    nc.vector.tensor_copy(st["rowsum_bf"][:], st["rowsum"][:])
                    nc.tensor.matmul(st["scores_ps"][:, 0:1], ones[:],
                                     st["rowsum_bf"][:], start=True, stop=True)
                elif stage == 2:
                    nc.vector.reciprocal(st["invs"][:], st["scores_ps"][:, 0:1])
                    nc.vector.tensor_scalar_mul(st["alpha"][:], st["p_sb"][:],
                                                st["invs"][:])
                elif stage in (3, 4):
                    h0 = (stage - 3) * (JT // 2)
                    jh = slice(h0, h0 + JT // 2)
                    nc.vector.tensor_mul(
                        st["o_sb"][:, jh, :], st["v_sb"][:, jh, :],
                        st["alpha"][:, jh, None].to_broadcast([P, JT // 2, D]))
                    nc.sync.dma_start(
                        out_ext[st["s"]].rearrange("(p j) d -> p j d", p=P)[:, jh, :],
                        st["o_sb"][:, jh, :])

            if pipeline:
                stage_pos = [min(1, NGRP - 1), min(2, NGRP - 1),
                             min(3, NGRP - 1), min(4, NGRP - 1),
                             min(6, NGRP - 1)]
            else:
                stage_pos = [-1] * 5
            for g in range(NGRP):
                if pending is not None:
                    for stg in range(5):
                        if stage_pos[stg] == g:
                            emit_softmax_stage(pending, stg)
                # [128, pair, 256]: per pair cols 0:128 = qT2, 128:256 = kT2
                qkT4 = chunks.tile([P, 4, 2 * P], BF16, tag="qkT4")
                for h in range(2):          # two 2-pair transpose blocks
                    pQK2 = ps_t.tile([P, 4 * P], BF16)
                    for b in range(2):      # pair within block
                        u = g * 4 + 2 * h + b
                        j0 = 2 * u
                        nc.tensor.transpose(
                            pQK2[:, 2 * b * P:(2 * b + 1) * P],
                            q_sb[:, j0:j0 + 2, :].rearrange("p a d -> p (a d)"),
                            ident[:])
                        nc.tensor.transpose(
                            pQK2[:, (2 * b + 1) * P:(2 * b + 2) * P],
                            k_sb[:, j0:j0 + 2, :].rearrange("p a d -> p (a d)"),
                            ident[:])
                    dst = qkT4[:, 2 * h:2 * h + 2, :].rearrange("p a c -> p (a c)")
                    if h == 0:
                        nc.vector.tensor_copy(dst, pQK2[:])
                    else:
                        nc.scalar.copy(dst, pQK2[:])
                pE4 = ps_e.tile([P, 4 * P], F32)
                nc.tensor.matmul(pE4.rearrange("p (a c) -> p a c", a=4),
                                 wblk[:], qkT4[:, :, 0:P],
                                 start=True, stop=False)
                nc.tensor.matmul(pE4.rearrange("p (a c) -> p a c", a=4),
                                 ublk[:], qkT4[:, :, P:2 * P],
                                 start=False, stop=True)
                tT4 = chunks.tile([P, 4 * P], BF16, tag="tT4")
                nc.scalar.activation(tT4[:], pE4[:],
                                     mybir.ActivationFunctionType.Tanh,
                                     bias=bias2[:], scale=1.0)
                for pr in range(4):
                    j0 = 2 * (g * 4 + pr)
                    nc.tensor.matmul(scores_ps[:, j0:j0 + 2],
                                     tT4[:, pr * P:(pr + 1) * P], vablk[:],
                                     start=True, stop=True)
                if cur is not None and g == NGRP // 2 - 1:
                    # final slice: exp the first half-row as soon as its
                    # scores exist, so only half the exp remains at the end
                    JT2 = JT // 2
                    nc.vector.tensor_add(cur["sm"][:, 0:JT2],
                                         scores_ps[:, 0:JT2],
                                         maskb_all[:, s, 0:JT2])
                    nc.scalar.activation(cur["p_sb"][:, 0:JT2],
                                         cur["sm"][:, 0:JT2],
                                         mybir.ActivationFunctionType.Exp,
                                         accum_out=cur["rowsum_a"][:])

            pending = cur if cur is not None else make_state()

            if not pipeline:
                for stage in range(5):
                    emit_softmax_stage(pending, stage)
                pending = None

        if pending is not None:
            # final slice: finish the second half-row exp, combine the two
            # half rowsums, then stream the v-scale + store in quarters so
            # the last output DMA overlaps the scaling
            st = pending
            JT2 = JT // 2
            if "rowsum_a" in st:
                # second half-row exp (first half ran inside the group loop),
                # combine half rowsums, reduce across partitions, normalize
                nc.vector.tensor_add(st["sm"][:, JT2:JT],
                                     st["scores_ps"][:, JT2:JT],
                                     maskb_all[:, st["s"], JT2:JT])
                nc.scalar.activation(st["p_sb"][:, JT2:JT],
                                     st["sm"][:, JT2:JT],
                                     mybir.ActivationFunctionType.Exp,
                                     accum_out=st["rowsum_b"][:])
                nc.vector.tensor_add(st["rowsum"][:], st["rowsum_a"][:],
                                     st["rowsum_b"][:])
                nc.vector.tensor_copy(st["rowsum_bf"][:], st["rowsum"][:])
                nc.tensor.matmul(st["scores_ps"][:, 0:1], ones[:],
                                 st["rowsum_bf"][:], start=True, stop=True)
                nc.vector.reciprocal(st["invs"][:], st["scores_ps"][:, 0:1])
                nc.vector.tensor_scalar_mul(st["alpha"][:], st["p_sb"][:],
                                            st["invs"][:])
            else:
                for stage in range(3):
                    emit_softmax_stage(pending, stage)
            jq = max(JT // 4, 1)
            for c0 in range(0, JT, jq):
                js = slice(c0, min(c0 + jq, JT))
                w = js.stop - js.start
                nc.vector.tensor_mul(
                    st["o_sb"][:, js, :], st["v_sb"][:, js, :],
                    st["alpha"][:, js, None].to_broadcast([P, w, D]))
                nc.sync.dma_start(
                    out_ext[st["s"]].rearrange("(p j) d -> p j d", p=P)[:, js, :],
                    st["o_sb"][:, js, :])

    nc.compile()
    return nc


def make_host_inputs(q, k, v, mask, Wa_w, Wa_b, Ua_w, Ua_b, va_w):
    """Returns per-core in_maps for the full problem."""
    q = np.ascontiguousarray(np.asarray(q, np.float32).reshape(B * H, L, D))
    k = np.ascontiguousarray(np.asarray(k, np.float32).reshape(B * H, L, D))
    v = np.ascontiguousarray(np.asarray(v, np.float32).reshape(B * H, L, D))
    # additive bias in device layout [slice, p, s-within-core merged later]
    JT = L // P
    mask = np.asarray(mask, np.int32).reshape(B * H, P, JT)
    maskb = np.where(mask == 0, np.float32(-10000.0), np.float32(0.0))

    bf16 = ml_dtypes.bfloat16
    WaT = np.asarray(Wa_w, np.float32).T  # [d, e]
    UaT = np.asarray(Ua_w, np.float32).T
    wblk = np.zeros((P, P), np.float32)
    wblk[0:D, 0:D] = WaT
    wblk[D:2 * D, D:2 * D] = WaT
    ublk = np.zeros((P, P), np.float32)
    ublk[0:D, 0:D] = UaT
    ublk[D:2 * D, D:2 * D] = UaT
    be = (np.asarray(Wa_b, np.float32) + np.asarray(Ua_b, np.float32))
    bias2 = np.concatenate([be, be]).reshape(P, 1)
    va = np.asarray(va_w, np.float32)[0]
    vablk = np.zeros((P, 2), np.float32)
    vablk[0:D, 0] = va
    vablk[D:2 * D, 1] = va
    ident = np.eye(P, dtype=np.float32)
    ones = np.ones((P, P), dtype=np.float32)

    wblk = wblk.astype(bf16)
    ublk = ublk.astype(bf16)
    vablk = vablk.astype(bf16)
    ident = ident.astype(bf16)
    ones = ones.astype(bf16)

    in_maps = []
    for i in range(N_CORES):
        sl = slice(i * SLICES_PER_CORE, (i + 1) * SLICES_PER_CORE)
        # [s, p, j] -> [p, s*j] so the device load is contiguous per partition
        mb = np.ascontiguousarray(
            maskb[sl].transpose(1, 0, 2).reshape(P, SLICES_PER_CORE * JT))
        in_maps.append({
            "q": q[sl], "k": k[sl], "v": v[sl], "maskb": mb,
            "wblk": wblk, "ublk": ublk, "bias2": bias2, "vablk": vablk,
            "ident": ident, "ones": ones,
        })
    return in_maps


_CACHED_NC = None


def kernel(q, k, v, mask, Wa_w, Wa_b, Ua_w, Ua_b, va_w, va_b=None, **kwargs):
    global _CACHED_NC
    if _CACHED_NC is None:
        _CACHED_NC = build_bass()
    in_maps = make_host_inputs(q, k, v, mask, Wa_w, Wa_b, Ua_w, Ua_b, va_w)
    res = run_bass_kernel_spmd(_CACHED_NC, in_maps, list(range(N_CORES)))
    out = np.concatenate([np.asarray(r["out"], np.float32) for r in res.results],
                         axis=0)
    return np.ascontiguousarray(out.reshape(B, H, L, D).astype(np.float32))


# revision 28
# speedup vs baseline: 1.1373x; 1.1373x over previous
"""Additive (Bahdanau) attention kernel for Trainium2, 8 NeuronCores.

Problem: B=4, H=16, L=8192, D=64 (fp32)
    e1 = q @ Wa_w.T + Wa_b ; e2 = k @ Ua_w.T + Ua_b
    t  = tanh(e1 + e2)
    e  = t @ va_w[0] + va_b          (va_b dropped: softmax shift-invariant)
    e  = where(mask == 0, -1e4, e)   (additive -1e4 bias before exp)
    alpha = softmax(e, axis=-1)      (over L)
    out = alpha[..., None] * v

Sharding: 64 independent (b, h) slices -> 8 per core, no collectives.

Per-slice layout: l = p * (L/128) + j with p = SBUF partition, j = tile
column; q/k/v live as [128, J, 64] (contiguous per partition), q/k/v are
cast to bf16 by the SWDGE DMA.  j-tiles are processed in PAIRS (transpose
q[:, j0:j0+2, :] [128,128] on TensorE -> bf16 psum rows (jj,d)), pairs
are processed in GROUPS of 4 to batch the weight matmuls (N=512) and
tanh.  Block-diagonal [WaT|WaT], [UaT|UaT] bf16 weights give e1+e2 for
both tiles of a pair in one K=128 contraction; tanh+bias on ScalarE ->
bf16; per-pair score matmul with tT stationary and a 2-column block-va
moving operand yields both score columns in natural [128, 1] layout.

Softmax: additive -1e4 mask bias (DVE), exp with fused per-row
accumulate (ScalarE accum_out), cross-partition sum via a ones-matmul on
TensorE (keeps GpSimd free for SWDGE descriptor generation), reciprocal
straight from PSUM, then a free-dim broadcast multiply with v on DVE.
The softmax/v-scale/store of each slice is software-pipelined into the
next slice's group loop (halves at g4/g6) so no engine blocks in-line.

Steady state is HBM-roofline bound (~7.1 MB HBM per slice).  All engine
queues are kept clear of long waits: consts are pre-cast to bf16 on the
host and loaded via HWDGE, the mask is one batched HWDGE load, v has 3
buffers so the SWDGE stream never stalls on a late v_sb recycle, and
PSUM->SBUF transpose copies alternate between DVE and ScalarE.
"""

import numpy as np
import ml_dtypes
from contextlib import ExitStack

import concourse.bass as bass
import concourse.tile as tile
from concourse import bacc, mybir
from concourse.bass_utils import run_bass_kernel_spmd

B, H, L, D = 4, 16, 8192, 64
N_CORES = 8
SLICES_PER_CORE = (B * H) // N_CORES
P = 128

F32 = mybir.dt.float32
BF16 = mybir.dt.bfloat16
I32 = mybir.dt.int32


def build_bass(n_slices=SLICES_PER_CORE, seq=L, pipeline=True):
    JT = seq // P            # j-columns per slice
    NPAIR = JT // 2          # tile pairs per slice
    NGRP = NPAIR // 4        # groups of 4 pairs
    assert NPAIR % 4 == 0

    nc = bacc.Bacc(target_bir_lowering=False)
    q_ext = nc.declare_dram_parameter("q", [n_slices, seq, D], F32, isOutput=False)
    k_ext = nc.declare_dram_parameter("k", [n_slices, seq, D], F32, isOutput=False)
    v_ext = nc.declare_dram_parameter("v", [n_slices, seq, D], F32, isOutput=False)
    # additive mask bias, precomputed on host in on-chip layout [p, s, j]
    # (a device-side gather of mask[s, (p j)] would be a 256B-descriptor
    # strided DMA that takes ~23us and poisons the DVE queue)
    mb_ext = nc.declare_dram_parameter("maskb", [P, n_slices * (seq // P)], F32,
                                       isOutput=False)
    wb_ext = nc.declare_dram_parameter("wblk", [P, P], BF16, isOutput=False)
    ub_ext = nc.declare_dram_parameter("ublk", [P, P], BF16, isOutput=False)
    b2_ext = nc.declare_dram_parameter("bias2", [P, 1], F32, isOutput=False)
    va_ext = nc.declare_dram_parameter("vablk", [P, 2], BF16, isOutput=False)
    id_ext = nc.declare_dram_parameter("ident", [P, P], BF16, isOutput=False)
    on_ext = nc.declare_dram_parameter("ones", [P, P], BF16, isOutput=False)
    out_ext = nc.declare_dram_parameter("out", [n_slices, seq, D], BF16,
                                        isOutput=True)

    with tile.TileContext(nc) as tc, ExitStack() as ctx:
        consts = ctx.enter_context(tc.tile_pool(name="consts", bufs=1))
        bigq = ctx.enter_context(tc.tile_pool(name="bigq", bufs=2))
        bigk = ctx.enter_context(tc.tile_pool(name="bigk", bufs=2))
        bigv = ctx.enter_context(tc.tile_pool(name="bigv", bufs=3))
        bigo = ctx.enter_context(tc.tile_pool(name="bigo", bufs=2))
        chunks = ctx.enter_context(tc.tile_pool(name="chunks", bufs=3))
        smalls = ctx.enter_context(tc.tile_pool(name="smalls", bufs=2))
        ps_t = ctx.enter_context(tc.tile_pool(name="ps_t", bufs=3, space="PSUM"))
        ps_e = ctx.enter_context(tc.tile_pool(name="ps_e", bufs=3, space="PSUM"))
        ps_sc = ctx.enter_context(tc.tile_pool(name="ps_sc", bufs=2, space="PSUM"))

        # chunk splits per slice: slice 0 starts fine-grained so the first
        # transposes can begin ~4us after the first descriptors, later
        # slices use big 2MB-HBM chunks for best SDMA efficiency
        jq8 = max(JT // 8, 1)
        first_splits = [(0, jq8), (jq8, 2 * jq8), (2 * jq8, JT // 2),
                        (JT // 2, JT)]
        rest_splits = [(0, JT // 2), (JT // 2, JT)]
        # the last slice tapers down so TensorE tracks chunk arrivals and
        # the final exposed compute is only ~one group deep
        last_splits = [(0, JT // 2), (JT // 2, 6 * jq8), (6 * jq8, 7 * jq8),
                       (7 * jq8, JT)]
        jc = first_splits[0][1]  # first-chunk width for the pre-issued load

        # first q/k chunk of slice 0 goes out before anything else so the
        # critical data feed starts immediately (consts ride HWDGE)
        q0_sb = bigq.tile([P, JT, D], BF16, name="q0_sb", tag="q_sb")
        k0_sb = bigk.tile([P, JT, D], BF16, name="k0_sb", tag="k_sb")
        nc.gpsimd.dma_start(q0_sb[:, 0:jc, :],
                            q_ext[0].rearrange("(p j) d -> p j d", p=P)[:, 0:jc, :])
        nc.gpsimd.dma_start(k0_sb[:, 0:jc, :],
                            k_ext[0].rearrange("(p j) d -> p j d", p=P)[:, 0:jc, :])

        # constants (pre-cast to bf16 on the host, loaded once via HWDGE)
        wblk = consts.tile([P, P], BF16)
        nc.sync.dma_start(wblk[:], wb_ext[:, :])
        ublk = consts.tile([P, P], BF16)
        nc.sync.dma_start(ublk[:], ub_ext[:, :])
        vablk = consts.tile([P, 2], BF16)
        nc.sync.dma_start(vablk[:], va_ext[:, :])
        bias2 = consts.tile([P, 1], F32)
        nc.sync.dma_start(bias2[:], b2_ext[:, :])
        ident = consts.tile([P, P], BF16)
        nc.sync.dma_start(ident[:], id_ext[:, :])
        ones = consts.tile([P, P], BF16)
        nc.sync.dma_start(ones[:], on_ext[:, :])
        # the whole additive mask bias in one contiguous load
        maskb_all = consts.tile([P, n_slices, JT], F32)
        nc.sync.dma_start(maskb_all[:],
                          mb_ext.rearrange("p (s j) -> p s j", s=n_slices))

        pending = None
        for s in range(n_slices):
            if s == 0:
                q_sb, k_sb = q0_sb, k0_sb
            else:
                q_sb = bigq.tile([P, JT, D], BF16, name="q_sb", tag="q_sb")
                k_sb = bigk.tile([P, JT, D], BF16, name="k_sb", tag="k_sb")
            splits = first_splits if s == 0 else rest_splits
            for ci, (j0, j1) in enumerate(splits):
                if s == 0 and ci == 0:
                    continue
                js = slice(j0, j1)
                nc.gpsimd.dma_start(
                    q_sb[:, js, :],
                    q_ext[s].rearrange("(p j) d -> p j d", p=P)[:, js, :])
                nc.gpsimd.dma_start(
                    k_sb[:, js, :],
                    k_ext[s].rearrange("(p j) d -> p j d", p=P)[:, js, :])
            v_sb = bigv.tile([P, JT, D], BF16)
            nc.gpsimd.dma_start(v_sb[:], v_ext[s].rearrange("(p j) d -> p j d", p=P))

            scores_ps = ps_sc.tile([P, JT], F32)

            def make_state(last=False):
                st = {
                    "s": s, "scores_ps": scores_ps, "v_sb": v_sb,
                    "sm": smalls.tile([P, JT], F32, tag="sm", name="sm"),
                    "p_sb": smalls.tile([P, JT], F32, tag="p_sb", name="p_sb"),
                    "rowsum": smalls.tile([P, 1], F32, tag="rowsum",
                                          name="rowsum"),
                    "rowsum_bf": smalls.tile([P, 1], BF16, tag="rowsum_bf",
                                             name="rowsum_bf"),
                    "invs": smalls.tile([P, 1], F32, tag="invs", name="invs"),
                    "alpha": smalls.tile([P, JT], BF16, tag="alpha",
                                         name="alpha"),
                    "o_sb": bigo.tile([P, JT, D], BF16, tag="o_sb",
                                      name="o_sb"),
                }
                if last:
                    st["rowsum_a"] = smalls.tile([P, 1], F32, tag="rowsum_a",
                                                 name="rowsum_a")
                    st["rowsum_b"] = smalls.tile([P, 1], F32, tag="rowsum_b",
                                                 name="rowsum_b")
                return st

            cur = None

            def emit_softmax_stage(st, stage):
                """Deferred softmax/v-scale/store for a previous slice."""
                if stage == 0:
                    nc.vector.tensor_add(st["sm"][:], st["scores_ps"][:],
                                         maskb_all[:, st["s"], :])
                    nc.scalar.activation(st["p_sb"][:], st["sm"][:],
                                         mybir.ActivationFunctionType.Exp,
                                         accum_out=st["rowsum"][:])
                elif stage == 1:
                    # cross-partition sum of rowsum via ones-matmul: every
                    # partition of scores_ps[:, 0:1] gets the full-row total
                    nc.vector.tensor_copy(st["rowsum_bf"][:], st["rowsum"][:])
                    nc.tensor.matmul(st["scores_ps"][:, 0:1], ones[:],
                                     st["rowsum_bf"][:], start=True, stop=True)
                elif stage == 2:
                    nc.vector.reciprocal(st["invs"][:], st["scores_ps"][:, 0:1])
                    nc.vector.tensor_scalar_mul(st["alpha"][:], st["p_sb"][:],
                                                st["invs"][:])
                elif stage in (3, 4):
                    h0 = (stage - 3) * (JT // 2)
                    jh = slice(h0, h0 + JT // 2)
                    nc.vector.tensor_mul(
                        st["o_sb"][:, jh, :], st["v_sb"][:, jh, :],
                        st["alpha"][:, jh, None].to_broadcast([P, JT // 2, D]))
                    nc.sync.dma_start(
                        out_ext[st["s"]].rearrange("(p j) d -> p j d", p=P)[:, jh, :],
                        st["o_sb"][:, jh, :])

            if pipeline:
                stage_pos = [min(1, NGRP - 1), min(2, NGRP - 1),
                             min(3, NGRP - 1), min(4, NGRP - 1),
                             min(6, NGRP - 1)]
            else:
                stage_pos = [-1] * 5
            for g in range(NGRP):
                if pending is not None:
                    for stg in range(5):
                        if stage_pos[stg] == g:
                            emit_softmax_stage(pending, stg)
                # [128, pair, 256]: per pair cols 0:128 = qT2, 128:256 = kT2
                qkT4 = chunks.tile([P, 4, 2 * P], BF16, tag="qkT4")
                for h in range(2):          # two 2-pair transpose blocks
                    pQK2 = ps_t.tile([P, 4 * P], BF16)
                    for b in range(2):      # pair within block
                        u = g * 4 + 2 * h + b
                        j0 = 2 * u
                        nc.tensor.transpose(
                            pQK2[:, 2 * b * P:(2 * b + 1) * P],
                            q_sb[:, j0:j0 + 2, :].rearrange("p a d -> p (a d)"),
                            ident[:])
                        nc.tensor.transpose(
                            pQK2[:, (2 * b + 1) * P:(2 * b + 2) * P],
                            k_sb[:, j0:j0 + 2, :].rearrange("p a d -> p (a d)"),
                            ident[:])
                    dst = qkT4[:, 2 * h:2 * h + 2, :].rearrange("p a c -> p (a c)")
                    if h == 0:
                        nc.vector.tensor_copy(dst, pQK2[:])
                    else:
                        nc.scalar.copy(dst, pQK2[:])
                pE4 = ps_e.tile([P, 4 * P], F32)
                nc.tensor.matmul(pE4.rearrange("p (a c) -> p a c", a=4),
                                 wblk[:], qkT4[:, :, 0:P],
                                 start=True, stop=False)
                nc.tensor.matmul(pE4.rearrange("p (a c) -> p a c", a=4),
                                 ublk[:], qkT4[:, :, P:2 * P],
                                 start=False, stop=True)
                tT4 = chunks.tile([P, 4 * P], BF16, tag="tT4")
                nc.scalar.activation(tT4[:], pE4[:],
                                     mybir.ActivationFunctionType.Tanh,
                                     bias=bias2[:], scale=1.0)
                for pr in range(4):
                    j0 = 2 * (g * 4 + pr)
                    nc.tensor.matmul(scores_ps[:, j0:j0 + 2],
                                     tT4[:, pr * P:(pr + 1) * P], vablk[:],
                                     start=True, stop=True)
                if cur is not None and g == NGRP // 2 - 1:
                    # final slice: exp the first half-row as soon as its
                    # scores exist, so only half the exp remains at the end
                    JT2 = JT // 2
                    nc.vector.tensor_add(cur["sm"][:, 0:JT2],
                                         scores_ps[:, 0:JT2],
                                         maskb_all[:, s, 0:JT2])
                    nc.scalar.activation(cur["p_sb"][:, 0:JT2],
                                         cur["sm"][:, 0:JT2],
                                         mybir.ActivationFunctionType.Exp,
                                         accum_out=cur["rowsum_a"][:])

            pending = cur if cur is not None else make_state()

            if not pipeline:
                for stage in range(5):
                    emit_softmax_stage(pending, stage)
                pending = None

        if pending is not None:
            # final slice: finish the second half-row exp, combine the two
            # half rowsums, then stream the v-scale + store in quarters so
            # the last output DMA overlaps the scaling
            st = pending
            JT2 = JT // 2
            if "rowsum_a" in st:
                # second half-row exp (first half ran inside the group loop),
                # combine half rowsums, reduce across partitions, normalize
                nc.vector.tensor_add(st["sm"][:, JT2:JT],
                                     st["scores_ps"][:, JT2:JT],
                                     maskb_all[:, st["s"], JT2:JT])
                nc.scalar.activation(st["p_sb"][:, JT2:JT],
                                     st["sm"][:, JT2:JT],
                                     mybir.ActivationFunctionType.Exp,
                                     accum_out=st["rowsum_b"][:])
                nc.vector.tensor_add(st["rowsum"][:], st["rowsum_a"][:],
                                     st["rowsum_b"][:])
                nc.vector.tensor_copy(st["rowsum_bf"][:], st["rowsum"][:])
                nc.tensor.matmul(st["scores_ps"][:, 0:1], ones[:],
                                 st["rowsum_bf"][:], start=True, stop=True)
                nc.vector.reciprocal(st["invs"][:], st["scores_ps"][:, 0:1])
                nc.vector.tensor_scalar_mul(st["alpha"][:], st["p_sb"][:],
                                            st["invs"][:])
            else:
                for stage in range(3):
                    emit_softmax_stage(pending, stage)
            jq = max(JT // 4, 1)
            for c0 in range(0, JT, jq):
                js = slice(c0, min(c0 + jq, JT))
                w = js.stop - js.start
                nc.vector.tensor_mul(
                    st["o_sb"][:, js, :], st["v_sb"][:, js, :],
                    st["alpha"][:, js, None].to_broadcast([P, w, D]))
                nc.sync.dma_start(
                    out_ext[st["s"]].rearrange("(p j) d -> p j d", p=P)[:, js, :],
                    st["o_sb"][:, js, :])

    nc.compile()
    return nc


def make_host_inputs(q, k, v, mask, Wa_w, Wa_b, Ua_w, Ua_b, va_w):
    """Returns per-core in_maps for the full problem."""
    q = np.ascontiguousarray(np.asarray(q, np.float32).reshape(B * H, L, D))
    k = np.ascontiguousarray(np.asarray(k, np.float32).reshape(B * H, L, D))
    v = np.ascontiguousarray(np.asarray(v, np.float32).reshape(B * H, L, D))
    # additive bias in device layout [slice, p, s-within-core merged later]
    JT = L // P
    mask = np.asarray(mask, np.int32).reshape(B * H, P, JT)
    maskb = np.where(mask == 0, np.float32(-10000.0), np.float32(0.0))

    bf16 = ml_dtypes.bfloat16
    WaT = np.asarray(Wa_w, np.float32).T  # [d, e]
    UaT = np.asarray(Ua_w, np.float32).T
    wblk = np.zeros((P, P), np.float32)
    wblk[0:D, 0:D] = WaT
    wblk[D:2 * D, D:2 * D] = WaT
    ublk = np.zeros((P, P), np.float32)
    ublk[0:D, 0:D] = UaT
    ublk[D:2 * D, D:2 * D] = UaT
    be = (np.asarray(Wa_b, np.float32) + np.asarray(Ua_b, np.float32))
    bias2 = np.concatenate([be, be]).reshape(P, 1)
    va = np.asarray(va_w, np.float32)[0]
    vablk = np.zeros((P, 2), np.float32)
    vablk[0:D, 0] = va
    vablk[D:2 * D, 1] = va
    ident = np.eye(P, dtype=np.float32)
    ones = np.ones((P, P), dtype=np.float32)

    wblk = wblk.astype(bf16)
    ublk = ublk.astype(bf16)
    vablk = vablk.astype(bf16)
    ident = ident.astype(bf16)
    ones = ones.astype(bf16)

    in_maps = []
    for i in range(N_CORES):
        sl = slice(i * SLICES_PER_CORE, (i + 1) * SLICES_PER_CORE)
        # [s, p, j] -> [p, s*j] so the device load is contiguous per partition
        mb = np.ascontiguousarray(
            maskb[sl].transpose(1, 0, 2).reshape(P, SLICES_PER_CORE * JT))
        in_maps.append({
            "q": q[sl], "k": k[sl], "v": v[sl], "maskb": mb,
            "wblk": wblk, "ublk": ublk, "bias2": bias2, "vablk": vablk,
            "ident": ident, "ones": ones,
        })
    return in_maps


_CACHED_NC = None


def kernel(q, k, v, mask, Wa_w, Wa_b, Ua_w, Ua_b, va_w, va_b=None, **kwargs):
    global _CACHED_NC
    if _CACHED_NC is None:
        _CACHED_NC = build_bass()
    in_maps = make_host_inputs(q, k, v, mask, Wa_w, Wa_b, Ua_w, Ua_b, va_w)
    res = run_bass_kernel_spmd(_CACHED_NC, in_maps, list(range(N_CORES)))
    out = np.concatenate([np.asarray(r["out"], np.float32) for r in res.results],
                         axis=0)
    return np.ascontiguousarray(out.reshape(B, H, L, D).astype(np.float32))
